# revision 6
# baseline (speedup 1.0000x reference)
"""AdaptivePointCNP on 8 Trainium2 NeuronCores.

Sharding: data-parallel over (batch b, query-block q): core d = b*4 + q handles
queries [q*512, (q+1)*512) of the 2048 concatenated (ctx+tgt) points of sample b.
Each core holds the full point cloud of its sample (needed for kNN source), per
the sharding hint. Global BatchNorm statistics (train-mode, over all B*N*k
elements) are reduced on host between stages (tiny 64-float vectors); neighbor
features h are exchanged between the 4 cores of a sample between conv layers.

All FLOPs (d2, top-k, gathers, weightnet MLPs, point-conv einsums, activations)
run on the NeuronCores; the host only slices/replicates inputs, reduces BN stat
vectors, concatenates per-core h blocks, and assembles the final output.
"""

import numpy as np
import jax
import jax.numpy as jnp
from jax import lax
from functools import partial

EPS = 1e-5
CMCO = 16
K_THETA = 300
K_CNN = 5
B, C, T, XD, YD = 2, 1024, 1024, 2, 1
N = C + T              # 2048 points per sample
NBLK = 4               # query blocks per sample
QB = N // NBLK         # 512 queries per core
NDEV = 8

_devices = None
_jits = {}
_dev_const = {}   # (dev_idx, name) -> device array, cached across calls


def _putc(d, name, arr):
    key = (d, name)
    if key not in _dev_const:
        _dev_const[key] = jax.device_put(arr, _devs()[d])
    return _dev_const[key]


def _devs():
    global _devices
    if _devices is None:
        _devices = jax.devices()[:NDEV]
    return _devices


# ---------------- per-device stage functions (jitted) ----------------

def _lin(x, w, b):
    return x @ w + b


def _bn_swish(x, g, be, mean, var):
    xh = (x - mean) * lax.rsqrt(var + EPS) * g + be
    return xh * jax.nn.sigmoid(xh)


def _stats(x):
    # partial sums over all but last axis -> (2, ch)
    s1 = jnp.sum(x, axis=tuple(range(x.ndim - 1)))
    s2 = jnp.sum(x * x, axis=tuple(range(x.ndim - 1)))
    return jnp.stack([s1, s2])


def stage1(coords, q0, p_theta_w1, p_theta_b1, pc_w1, pc_b1):
    """coords: (N,2) full sample. q0: scalar query offset.
    Returns idx300 (QB,300), deltas (QB,300,2), x1t (QB,300,32),
    x1c (4,QB,5,32), stat partials."""
    q = lax.dynamic_slice(coords, (q0, 0), (QB, XD))          # (QB,2)
    d2 = jnp.sum(jnp.square(q[:, None, :] - coords[None, :, :]), -1)  # (QB,N)
    _, idx = lax.top_k(-d2, K_THETA)                          # (QB,300)
    nb_c = coords[idx]                                        # (QB,300,2)
    deltas = nb_c - q[:, None, :]
    x1t = deltas @ p_theta_w1 + p_theta_b1                    # (QB,300,32)
    d5 = deltas[:, :K_CNN]                                    # (QB,5,2)
    x1c = jnp.einsum('skd,ldh->lskh', d5, pc_w1) + pc_b1[:, None, None]  # (4,QB,5,32)
    st = jnp.concatenate([_stats(x1t)[None], jax.vmap(_stats)(x1c)], 0)  # (5,2,32)
    return idx, idx[:, :K_CNN], deltas, x1t, x1c, st


def stage2(x1t, x1c, g1t, be1t, m1t, v1t, g1c, be1c, m1c, v1c,
           w2t, b2t, w2c, b2c):
    h1t = _bn_swish(x1t, g1t, be1t, m1t, v1t)
    x2t = h1t @ w2t + b2t
    h1c = _bn_swish(x1c, g1c[:, None, None], be1c[:, None, None],
                    m1c[:, None, None], v1c[:, None, None])
    x2c = jnp.einsum('lskh,lhj->lskj', h1c, w2c) + b2c[:, None, None]
    st = jnp.concatenate([_stats(x2t)[None], jax.vmap(_stats)(x2c)], 0)
    return x2t, x2c, st


def stage3(x2t, x2c, g2t, be2t, m2t, v2t, g2c, be2c, m2c, v2c,
           w3t, b3t, w3c, b3c):
    h2t = _bn_swish(x2t, g2t, be2t, m2t, v2t)
    x3t = h2t @ w3t + b3t                                     # (QB,300,16)
    h2c = _bn_swish(x2c, g2c[:, None, None], be2c[:, None, None],
                    m2c[:, None, None], v2c[:, None, None])
    x3c = jnp.einsum('lskh,lhj->lskj', h2c, w3c) + b3c[:, None, None]
    st = jnp.concatenate([_stats(x3t)[None], jax.vmap(_stats)(x3c)], 0)
    return x3t, x3c, st


def stage4(x3t, x3c, idx, ctx_vals,
           g3t, be3t, m3t, v3t, g3c, be3c, m3c, v3c, Wlt, blt):
    wt = _bn_swish(x3t, g3t, be3t, m3t, v3t)                  # (QB,300,16)
    wc = _bn_swish(x3c, g3c[:, None, None], be3c[:, None, None],
                   m3c[:, None, None], v3c[:, None, None])    # (4,QB,5,16)
    dens = (idx < C).astype(jnp.float32)                      # (QB,300)
    sig_full = jnp.concatenate([ctx_vals[:, 0], jnp.zeros((T,), jnp.float32)])
    sig = sig_full[idx]                                       # (QB,300)
    dp = jnp.einsum('sk,skm->sm', dens, wt) / K_THETA         # (QB,16)
    sp = jnp.einsum('sk,skm->sm', sig, wt) / K_THETA
    dp = dp @ Wlt + blt                                       # (QB,128)
    sp = sp @ Wlt + blt
    h0 = jnp.concatenate([dp, sp], -1)                        # (QB,256)
    return h0, wc[0], wc[1], wc[2], wc[3]


def stage_conv(h_full, idx5, wc_l, Wl, bl, relu):
    nb = h_full[idx5]                                         # (QB,5,ci)
    pc = jnp.einsum('skc,skm->scm', nb, wc_l) / K_CNN         # (QB,ci,16)
    out = pc.reshape(QB, -1) @ Wl + bl
    if relu:
        out = jax.nn.relu(out)
    return out


def stage_final(h_full, idx5, wc_l, Wl, bl):
    f = stage_conv(h_full, idx5, wc_l, Wl, bl, False)         # (QB,2)
    # softplus part 1: exp(-|x|) (part 2 runs in a separate module — the
    # neuron tensorizer crashes on a fused exp->log activation chain)
    e = jnp.exp(-jnp.abs(f[:, 1]))
    return f, e


def softplus_fin(f, e):
    return f[:, 0], jnp.maximum(f[:, 1], 0.0) + jnp.log(1.0 + e)


def _jit(name, fn, **kw):
    if name not in _jits:
        _jits[name] = jax.jit(fn, **kw)
    return _jits[name]


def _reduce_stats(parts, count):
    """parts: list of (5,2,32) np arrays -> (mean,var) arrays (5,32)."""
    s = np.sum(np.stack(parts), 0)
    mean = s[:, 0] / count
    var = s[:, 1] / count - mean * mean
    return mean.astype(np.float32), np.maximum(var, 0).astype(np.float32)


def kernel(ctx_coords, ctx_values, tgt_coords, params):
    devs = _devs()
    pt = params['theta']
    pcs = [params[k] for k in ('c1', 'c2', 'c3', 'c4')]

    coords = np.concatenate([np.asarray(ctx_coords), np.asarray(tgt_coords)], 1)  # (B,N,2)

    # stack the 4 c-layer weightnet params (they share shapes)
    cw = {k: np.stack([np.asarray(p[k]) for p in pcs]) for k in
          ('w1', 'b1', 'g1', 'be1', 'w2', 'b2', 'g2', 'be2', 'w3', 'b3', 'g3', 'be3')}

    # ---- place per-device inputs
    dev_in = []
    for d in range(NDEV):
        b, q = d // NBLK, d % NBLK
        put = partial(jax.device_put, device=devs[d])
        dev_in.append(dict(
            coords=put(coords[b]),
            ctx_vals=put(np.asarray(ctx_values[b])),
            q0=q * QB,
        ))

    f32 = np.float32

    # ---- stage 1
    s1 = _jit('s1', stage1, static_argnums=(1,))
    r1 = []
    for d in range(NDEV):
        di = dev_in[d]
        r1.append(s1(di['coords'], di['q0'],
                     _putc(d, 'tw1', np.asarray(pt['w1'])), _putc(d, 'tb1', np.asarray(pt['b1'])),
                     _putc(d, 'cw1', cw['w1']), _putc(d, 'cb1', cw['b1'])))
    st_parts = [np.asarray(r[5]) for r in r1]
    cnt_t = B * N * K_THETA
    cnt_c = B * N * K_CNN
    m1, v1 = _reduce_stats(st_parts, np.array([cnt_t] + [cnt_c] * 4)[:, None])

    # ---- stage 2
    s2 = _jit('s2', stage2)
    r2 = []
    for d in range(NDEV):
        pw = partial(jax.device_put, device=devs[d])
        r2.append(s2(r1[d][3], r1[d][4],
                     _putc(d, 'tg1', np.asarray(pt['g1'])), _putc(d, 'tbe1', np.asarray(pt['be1'])),
                     pw(m1[0]), pw(v1[0]),
                     _putc(d, 'cg1', cw['g1']), _putc(d, 'cbe1', cw['be1']), pw(m1[1:]), pw(v1[1:]),
                     _putc(d, 'tw2', np.asarray(pt['w2'])), _putc(d, 'tb2', np.asarray(pt['b2'])),
                     _putc(d, 'cw2', cw['w2']), _putc(d, 'cb2', cw['b2'])))
    m2, v2 = _reduce_stats([np.asarray(r[2]) for r in r2],
                           np.array([cnt_t] + [cnt_c] * 4)[:, None])

    # ---- stage 3
    s3 = _jit('s3', stage3)
    r3 = []
    for d in range(NDEV):
        pw = partial(jax.device_put, device=devs[d])
        r3.append(s3(r2[d][0], r2[d][1],
                     _putc(d, 'tg2', np.asarray(pt['g2'])), _putc(d, 'tbe2', np.asarray(pt['be2'])),
                     pw(m2[0]), pw(v2[0]),
                     _putc(d, 'cg2', cw['g2']), _putc(d, 'cbe2', cw['be2']), pw(m2[1:]), pw(v2[1:]),
                     _putc(d, 'tw3', np.asarray(pt['w3'])), _putc(d, 'tb3', np.asarray(pt['b3'])),
                     _putc(d, 'cw3', cw['w3']), _putc(d, 'cb3', cw['b3'])))
    m3, v3 = _reduce_stats([np.asarray(r[2]) for r in r3],
                           np.array([cnt_t] + [cnt_c] * 4)[:, None])

    # ---- stage 4: finish weightnets, theta point-conv -> h0 blocks
    s4 = _jit('s4', stage4)
    r4 = []
    for d in range(NDEV):
        pw = partial(jax.device_put, device=devs[d])
        r4.append(s4(r3[d][0], r3[d][1], r1[d][0], dev_in[d]['ctx_vals'],
                     _putc(d, 'tg3', np.asarray(pt['g3'])), _putc(d, 'tbe3', np.asarray(pt['be3'])),
                     pw(m3[0]), pw(v3[0]),
                     _putc(d, 'cg3', cw['g3']), _putc(d, 'cbe3', cw['be3']), pw(m3[1:]), pw(v3[1:]),
                     _putc(d, 'tWl', np.asarray(pt['Wl'])), _putc(d, 'tbl', np.asarray(pt['bl']))))

    idx5 = [r1[d][1] for d in range(NDEV)]

    # ---- conv layers c1..c3 with host h-exchange within each sample group
    sc = _jit('sc', stage_conv, static_argnums=(5,))
    h_blocks = [np.asarray(r4[d][0]) for d in range(NDEV)]    # (QB,256) each

    for li in range(3):
        h_full = [np.concatenate(h_blocks[b * NBLK:(b + 1) * NBLK], 0) for b in range(B)]
        new_blocks = []
        for d in range(NDEV):
            b = d // NBLK
            pw = partial(jax.device_put, device=devs[d])
            wc_l = r4[d][1 + li]                              # (QB,5,16) on device
            p = pcs[li]
            out = sc(pw(h_full[b].astype(f32)), idx5[d], wc_l,
                     _putc(d, f'cWl{li}', np.asarray(p['Wl'])),
                     _putc(d, f'cbl{li}', np.asarray(p['bl'])), True)
            new_blocks.append(out)
        h_blocks = [np.asarray(o) for o in new_blocks]

    # ---- final conv c4
    sf = _jit('sf', stage_final)
    spf = _jit('spf', softplus_fin)
    h_full = [np.concatenate(h_blocks[b * NBLK:(b + 1) * NBLK], 0) for b in range(B)]
    outs = []
    for d in range(NDEV):
        b = d // NBLK
        pw = partial(jax.device_put, device=devs[d])
        p = pcs[3]
        fr, e = sf(pw(h_full[b].astype(f32)), idx5[d], r4[d][4],
                   _putc(d, 'cWl3', np.asarray(p['Wl'])),
                   _putc(d, 'cbl3', np.asarray(p['bl'])))
        outs.append(spf(fr, e))

    f_mu_blocks = [np.asarray(o[0]) for o in outs]
    f_sg_blocks = [np.asarray(o[1]) for o in outs]
    fmu = np.stack([np.concatenate(f_mu_blocks[b * NBLK:(b + 1) * NBLK]) for b in range(B)])
    fsg = np.stack([np.concatenate(f_sg_blocks[b * NBLK:(b + 1) * NBLK]) for b in range(B)])

    f_mu = fmu[:, C:].astype(f32)                             # (B,T)
    f_sigma = fsg[:, C:].astype(f32)
    sigma = f_sigma[:, :, None] * np.eye(T, dtype=f32)[None]  # (B,T,T)
    return f_mu, sigma
